# revision 8
# baseline (speedup 1.0000x reference)
"""NodeConv kernel for 8 Trainium2 NeuronCores.

Reference computes, for adj [B,1,N,N], node [B,nin,N], Wi/Wj [nout,nin]:
    x  = node[:, :, None, :] * adj          # [B,nin,N,N]
    yi = einsum('oc,bcij->boij', Wi, x)
    yj = einsum('oc,bcij->boij', Wj, x)
    out = I * yi + (1-I) * yj

Because adj[b,i,j] does not depend on the contraction channel c, the
contraction factors out:
    off-diag: out[b,o,i,j] = adj[b,i,j] * (Wj @ node[b])[o,j]
    diag:     out[b,o,j,j] = adj[b,j,j] * (Wi @ node[b])[o,j]

So per batch we need one tiny matmul u = Wj@node, a broadcast multiply
out[o,i,j] = adj[i,j]*u[o,j], and a diagonal patch with
dv = Wi @ (node[:, :128] * adj_diag) (the diag scaling is folded into
the host-prepared input, so dv is a single matmul).

The 128 MiB output write is the memory roofline; per core the 16 MiB
store drains at ~425 GB/s (~39.5 us) with the 16 HW queues saturated,
so total time = (time until the first store hits the queues) + 39.5 us.
Lead-in minimization:
  - adj ships as ONE bf16 term (2^-9 relative error, gate is 2e-2),
    so the pk input load is 128 KiB instead of 384 KiB
  - ckf is all-bf16 (node/weights as 2-term bf16 splits, u error
    ~2^-17): 256 KiB, and its u-matmul chain is 3 fast bf16 matmuls
  - ckf loads on the sync HWDGE ring while pk loads on the scalar ring
  - chunk 0 is processed in four 512-column units and chunk 1 in two
    1024-column units, EACH with its own PSUM tile + SBUF tile (the
    tile framework tracks dependencies per tile, not per subrange), so
    the first 256 KiB store is in flight ~6 us earlier than the
    baseline's first 2 MiB group store

Sharding: core c handles batch b=c//2, row half h=c%2 (128 rows). Odd
halves get their columns rolled by -128 on the host so the diagonal of
local row l sits at local column l on every core -> one SPMD program;
the host rolls the output back while gathering.

Per-core device program:
  - u = Wj @ node_r (PE, 3 accumulating bf16 matmuls), DVE copies it
    to SBUF; dv = Wi @ nd (1 bf16 matmul), ScalarE copies it
  - per store unit (chunk p, cols [c0,c0+w)): PE broadcasts the adj
    rows to all 128 partitions with one-hot-selector matmuls (one per
    512 cols / PSUM bank); DVE multiplies by u broadcast along the row
    dim; ScalarE patches the diagonal elements via a stride-257 view;
    the store alternates between the two HWDGE rings.
    NODECONV_LAYOUT=po stores chunk-major (contiguous DRAM blocks) and
    the host transposes while gathering.
"""

import os

import numpy as np

NCORES = 8
B, N, NIN, NOUT = 4, 256, 128, 128
RPC = 128          # rows per core
CH = 16            # chunks per core
RCH = 8            # rows per chunk
FREE = RCH * N     # 2048 free elems per chunk

NTERMS = int(os.environ.get("NODECONV_NTERMS", "1"))   # bf16 terms (1, 2 or 3)
OUT_BUFS = int(os.environ.get("NODECONV_OUT_BUFS", "4"))
LAYOUT = os.environ.get("NODECONV_LAYOUT", "oc")       # oc: [NOUT, RPC*N]; po: chunk-major
# how many store units the first chunks split into ("4,2": chunk0 -> 4x512,
# chunk1 -> 2x1024, rest whole)
SPLIT = [int(x) for x in os.environ.get("NODECONV_SPLIT", "4,2").split(",") if x]

KP = CH * NTERMS   # contraction partitions of the broadcast matmul
CKF_COLS = 2 * N + 4 * NOUT   # n0 n1 | wj0 wj1 | wi0 | nd0

_cached = {}

last_results = None  # BassKernelResults of the most recent kernel() call


def _units():
    """(chunk, col0, width) store units; early chunks split finer."""
    units = []
    for p in range(CH):
        nsub = SPLIT[p] if p < len(SPLIT) else 1
        w = FREE // nsub
        for s in range(nsub):
            units.append((p, s * w, w))
    return units


def _build_nc():
    key = (NTERMS, OUT_BUFS, LAYOUT, tuple(SPLIT))
    if key in _cached:
        return _cached[key]

    from contextlib import ExitStack

    import concourse.tile as tile
    from concourse import bacc, mybir

    f32 = mybir.dt.float32
    bf16 = mybir.dt.bfloat16

    nc = bacc.Bacc(
        "TRN2", target_bir_lowering=False, debug=False, num_devices=NCORES
    )

    # pk: [KP, 2*FREE] bf16 — adj terms in [:, :FREE], one-hot selector
    # blocks in [:, FREE:]
    pk = nc.dram_tensor("pk", [KP, 2 * FREE], bf16, kind="ExternalInput").ap()
    # ckf: [128, 1024] bf16 — node terms | WjT terms | WiT | diag-scaled node
    ckf = nc.dram_tensor("ckf", [NIN, CKF_COLS], bf16, kind="ExternalInput").ap()
    if LAYOUT == "po":
        out = nc.dram_tensor("out", [CH * NOUT, FREE], f32, kind="ExternalOutput").ap()
    else:
        out = nc.dram_tensor("out", [NOUT, RPC * N], f32, kind="ExternalOutput").ap()

    with tile.TileContext(nc) as tc, ExitStack() as ctx:
        const = ctx.enter_context(tc.tile_pool(name="const", bufs=1))
        psum = ctx.enter_context(tc.tile_pool(name="psum", bufs=2, space="PSUM"))
        outp = ctx.enter_context(tc.tile_pool(name="outp", bufs=OUT_BUFS))

        # ckf (sync ring) and pk (scalar ring) load concurrently; the u
        # matmul chain off ckf is the critical path to the first store.
        ckf_sb = const.tile([NIN, CKF_COLS], bf16)
        nc.sync.dma_start(out=ckf_sb[:], in_=ckf)
        pk_sb = const.tile([KP, 2 * FREE], bf16)
        nc.scalar.dma_start(out=pk_sb[:], in_=pk)

        n0 = ckf_sb[:, 0:N]
        n1 = ckf_sb[:, N : 2 * N]
        wj0 = ckf_sb[:, 2 * N : 2 * N + NOUT]
        wj1 = ckf_sb[:, 2 * N + NOUT : 2 * N + 2 * NOUT]
        wi0 = ckf_sb[:, 2 * N + 2 * NOUT : 2 * N + 3 * NOUT]
        nd0 = ckf_sb[:, 2 * N + 3 * NOUT : 2 * N + 4 * NOUT]

        # u = Wj @ node_r -> [nout, N], via 2-term bf16 splits (err ~2^-17)
        ps_u = psum.tile([NOUT, N], f32, tag="mm")
        nc.tensor.matmul(ps_u[:], lhsT=wj0, rhs=n0, start=True, stop=False)
        nc.tensor.matmul(ps_u[:], lhsT=wj0, rhs=n1, start=False, stop=False)
        nc.tensor.matmul(ps_u[:], lhsT=wj1, rhs=n0, start=False, stop=True)
        u_sb = const.tile([NOUT, N], f32)
        nc.vector.tensor_copy(u_sb[:], ps_u[:])

        # dv[o,l] = adj_diag[l] * (Wi @ node_r)[o,l]; the diag scaling is
        # folded into nd on the host, so this is one matmul + copy.
        ps_dv = psum.tile([NOUT, RPC], f32, tag="mm")
        nc.tensor.matmul(ps_dv[:], lhsT=wi0, rhs=nd0, start=True, stop=True)
        dv_sb = const.tile([NOUT, RPC], f32)
        nc.scalar.copy(dv_sb[:], ps_dv[:])

        def patch(o_ap, base, p, c0, w):
            # diagonal of local row l=8p+k sits at chunk-free offset 8p+k*257
            k0 = max(0, -(-(c0 - RCH * p) // 257))
            k1 = min(RCH - 1, (c0 + w - 1 - RCH * p) // 257)
            if k0 <= k1:
                nc.scalar.copy(
                    o_ap[
                        :,
                        base + RCH * p + 257 * k0 - c0 : base
                        + RCH * p
                        + 257 * k1
                        - c0
                        + 1 : 257,
                    ],
                    dv_sb[:, RCH * p + k0 : RCH * p + k1 + 1],
                )

        def dst_of(p, c0, w):
            if LAYOUT == "po":
                return out[NOUT * p : NOUT * (p + 1), c0 : c0 + w]
            return out[:, FREE * p + c0 : FREE * p + c0 + w]

        ui = 0
        # fine-grained early chunks: per-unit PSUM + SBUF tiles so the first
        # store leaves as soon as the first 512 columns are multiplied
        fine = [(p, c0, w) for p, c0, w in _units() if p < len(SPLIT)]
        nfine = len({p for p, _, _ in fine})
        for p, c0, w in fine:
            ps = psum.tile([NOUT, w], f32, tag="mm", name=f"ps_{p}_{c0}")
            lhs = pk_sb[:, FREE + NOUT * p : FREE + NOUT * (p + 1)]
            for q in range(w // 512):
                nc.tensor.matmul(
                    ps[:, 512 * q : 512 * (q + 1)],
                    lhsT=lhs,
                    rhs=pk_sb[:, c0 + 512 * q : c0 + 512 * (q + 1)],
                    start=True,
                    stop=True,
                )
            o_sb = outp.tile([NOUT, w], f32, tag="osb_s", bufs=6, name=f"o_{p}_{c0}")
            k = w // N
            u_rep = u_sb[:].unsqueeze(1).broadcast_to([NOUT, k, N])
            nc.vector.tensor_mul(
                o_sb[:].rearrange("p (k j) -> p k j", k=k),
                ps[:].rearrange("p (k j) -> p k j", k=k),
                u_rep,
            )
            patch(o_sb, 0, p, c0, w)
            eng = nc.sync if ui % 2 == 0 else nc.scalar
            eng.dma_start(out=dst_of(p, c0, w), in_=o_sb[:])
            ui += 1

        # steady state: identical to the proven baseline — per-chunk PSUM
        # tiles, 2-chunk (2 MiB) group stores alternating HWDGE rings
        u_rep8 = u_sb[:].unsqueeze(1).broadcast_to([NOUT, RCH, N])
        p = nfine
        while p < CH:
            gsz = min(2, CH - p)
            o_sb = outp.tile(
                [NOUT, gsz * FREE], f32, tag="osb", name=f"o_g{p}"
            )
            p0 = p
            for g in range(gsz):
                ps_b = psum.tile([NOUT, FREE], f32, tag="mm", name=f"ps_b{p}")
                lhs = pk_sb[:, FREE + NOUT * p : FREE + NOUT * (p + 1)]
                for q in range(FREE // 512):
                    sl = slice(512 * q, 512 * (q + 1))
                    nc.tensor.matmul(
                        ps_b[:, sl], lhsT=lhs, rhs=pk_sb[:, sl], start=True, stop=True
                    )
                nc.vector.tensor_mul(
                    o_sb[:, g * FREE : (g + 1) * FREE].rearrange(
                        "p (k j) -> p k j", k=RCH
                    ),
                    ps_b[:].rearrange("p (k j) -> p k j", k=RCH),
                    u_rep8,
                )
                patch(o_sb, g * FREE, p, 0, FREE)
                p += 1
            eng = nc.sync if ui % 2 == 0 else nc.scalar
            if LAYOUT == "po":
                dst = out[NOUT * p0 : NOUT * p, 0:FREE].rearrange(
                    "(g o) w -> o g w", g=gsz
                )
                eng.dma_start(
                    out=dst, in_=o_sb[:].rearrange("p (g w) -> p g w", g=gsz)
                )
            else:
                eng.dma_start(out=out[:, FREE * p0 : FREE * p], in_=o_sb[:])
            ui += 1

    nc.compile()
    _cached[key] = nc
    return nc


def _split_terms(x, nterms):
    """Split fp32 array into bf16 terms whose fp32 sum approximates x.
    1 term has <=2^-9 relative error, 2 terms <=2^-18, 3 terms exact."""
    import ml_dtypes

    terms = []
    r = x
    for _ in range(nterms):
        t = r.astype(ml_dtypes.bfloat16)
        terms.append(t)
        r = (r - t.astype(np.float32)).astype(np.float32)
    return terms


def _in_maps(adj, node, Wi, Wj):
    import ml_dtypes

    bf16 = ml_dtypes.bfloat16
    sel = np.zeros((KP, CH * NOUT), bf16)
    for p in range(CH):
        for t in range(NTERMS):
            sel[CH * t + p, NOUT * p : NOUT * (p + 1)] = 1.0
    wj_t = _split_terms(Wj.T, 2)
    wi0 = Wi.T.astype(bf16)
    maps = []
    for c in range(NCORES):
        b, h = divmod(c, 2)
        r0 = RPC * h
        a = adj[b, 0, r0 : r0 + RPC, :]
        diag_row = a[np.arange(RPC), r0 + np.arange(RPC)]
        if h:
            ar = np.roll(a, -r0, axis=1)
            noder = np.roll(node[b], -r0, axis=1)
        else:
            ar = a
            noder = node[b]
        pk = np.empty((KP, 2 * FREE), bf16)
        terms = _split_terms(ar.reshape(CH, FREE), NTERMS)
        for t in range(NTERMS):
            pk[CH * t : CH * (t + 1), 0:FREE] = terms[t]
        pk[:, FREE:] = sel
        n_t = _split_terms(noder, 2)
        ckf = np.empty((NIN, CKF_COLS), bf16)
        ckf[:, 0:N] = n_t[0]
        ckf[:, N : 2 * N] = n_t[1]
        ckf[:, 2 * N : 2 * N + NOUT] = wj_t[0]
        ckf[:, 2 * N + NOUT : 2 * N + 2 * NOUT] = wj_t[1]
        ckf[:, 2 * N + 2 * NOUT : 2 * N + 3 * NOUT] = wi0
        ckf[:, 2 * N + 3 * NOUT :] = (
            noder[:, 0:RPC] * diag_row[None, :].astype(np.float32)
        ).astype(bf16)
        maps.append({"pk": pk, "ckf": ckf})
    return maps


def kernel(**inputs):
    global last_results
    adj = np.asarray(inputs["adj"], dtype=np.float32)
    node = np.asarray(inputs["node"], dtype=np.float32)
    Wi = np.asarray(inputs["Wi"], dtype=np.float32)
    Wj = np.asarray(inputs["Wj"], dtype=np.float32)

    from concourse.bass_utils import run_bass_kernel_spmd

    nc = _build_nc()
    res = run_bass_kernel_spmd(nc, _in_maps(adj, node, Wi, Wj), list(range(NCORES)))
    last_results = res

    out = np.empty((B, NOUT, N, N), np.float32)
    for c in range(NCORES):
        b, h = divmod(c, 2)
        co = res.results[c]["out"]
        if LAYOUT == "po":
            co = np.ascontiguousarray(
                co.reshape(CH, NOUT, RCH, N).transpose(1, 0, 2, 3)
            ).reshape(NOUT, RPC, N)
        else:
            co = co.reshape(NOUT, RPC, N)
        if h:
            co = np.roll(co, RPC * h, axis=2)
        out[b, :, RPC * h : RPC * (h + 1), :] = co
    return out


# revision 12
# speedup vs baseline: 1.0440x; 1.0440x over previous
"""NodeConv kernel for 8 Trainium2 NeuronCores.

Reference computes, for adj [B,1,N,N], node [B,nin,N], Wi/Wj [nout,nin]:
    x  = node[:, :, None, :] * adj          # [B,nin,N,N]
    yi = einsum('oc,bcij->boij', Wi, x)
    yj = einsum('oc,bcij->boij', Wj, x)
    out = I * yi + (1-I) * yj

Because adj[b,i,j] does not depend on the contraction channel c, the
contraction factors out:
    off-diag: out[b,o,i,j] = adj[b,i,j] * (Wj @ node[b])[o,j]
    diag:     out[b,o,j,j] = adj[b,j,j] * (Wi @ node[b])[o,j]

So per batch we need one tiny matmul u = Wj@node, a broadcast multiply
out[o,i,j] = adj[i,j]*u[o,j], and a diagonal patch with
dv = Wi @ (node[:, :128] * adj_diag) (the diag scaling is folded into
the host-prepared input, so dv is a single matmul).

The 128 MiB output write is the memory roofline; per core the 16 MiB
store drains at ~425 GB/s (~39.5 us) with the 16 HW queues saturated,
so total time = (time until the first store hits the queues) + 39.5 us.
Lead-in minimization:
  - adj ships as ONE bf16 term (2^-9 relative error, gate is 2e-2),
    so the pk input load is 128 KiB instead of 384 KiB
  - ckf is all-bf16 (node/weights as 2-term bf16 splits, u error
    ~2^-17): 256 KiB, and its u-matmul chain is 3 fast bf16 matmuls
  - ckf loads on the sync HWDGE ring while pk loads on the scalar ring
  - chunk 0 is processed in four 512-column units and chunk 1 in two
    1024-column units, EACH with its own PSUM tile + SBUF tile (the
    tile framework tracks dependencies per tile, not per subrange), so
    the first 256 KiB store is in flight ~6 us earlier than the
    baseline's first 2 MiB group store

Sharding: core c handles batch b=c//2, row half h=c%2 (128 rows). Odd
halves get their columns rolled by -128 on the host so the diagonal of
local row l sits at local column l on every core -> one SPMD program;
the host rolls the output back while gathering.

Per-core device program:
  - u = Wj @ node_r (PE, 3 accumulating bf16 matmuls), DVE copies it
    to SBUF; dv = Wi @ nd (1 bf16 matmul), ScalarE copies it
  - per store unit (chunk p, cols [c0,c0+w)): PE broadcasts the adj
    rows to all 128 partitions with one-hot-selector matmuls (one per
    512 cols / PSUM bank); DVE multiplies by u broadcast along the row
    dim; ScalarE patches the diagonal elements via a stride-257 view;
    the store alternates between the two HWDGE rings.
    NODECONV_LAYOUT=po stores chunk-major (contiguous DRAM blocks) and
    the host transposes while gathering.
"""

import os

import numpy as np

NCORES = 8
B, N, NIN, NOUT = 4, 256, 128, 128
RPC = 128          # rows per core
CH = 16            # chunks per core
RCH = 8            # rows per chunk
FREE = RCH * N     # 2048 free elems per chunk

NTERMS = int(os.environ.get("NODECONV_NTERMS", "1"))   # bf16 terms (1, 2 or 3)
OUT_BUFS = int(os.environ.get("NODECONV_OUT_BUFS", "4"))
LAYOUT = os.environ.get("NODECONV_LAYOUT", "oc")       # oc: [NOUT, RPC*N]; po: chunk-major
# how many store units the first chunks split into ("4,2": chunk0 -> 4x512,
# chunk1 -> 2x1024, rest whole)
SPLIT = [int(x) for x in os.environ.get("NODECONV_SPLIT", "4,2").split(",") if x]
# bytes of dummy DRAM input declared before `out` (shifts its placement)
PAD_IN = int(os.environ.get("NODECONV_PAD_IN", "0"))

KP = CH * NTERMS   # contraction partitions of the broadcast matmul
CKF_COLS = 2 * N + 4 * NOUT   # n0 n1 | wj0 wj1 | wi0 | nd0

_cached = {}

last_results = None  # BassKernelResults of the most recent kernel() call


def _units():
    """(chunk, col0, width) store units; early chunks split finer."""
    units = []
    for p in range(CH):
        nsub = SPLIT[p] if p < len(SPLIT) else 1
        w = FREE // nsub
        for s in range(nsub):
            units.append((p, s * w, w))
    return units


def _build_nc():
    key = (NTERMS, OUT_BUFS, LAYOUT, tuple(SPLIT), PAD_IN)
    if key in _cached:
        return _cached[key]

    from contextlib import ExitStack

    import concourse.tile as tile
    from concourse import bacc, mybir

    f32 = mybir.dt.float32
    bf16 = mybir.dt.bfloat16

    nc = bacc.Bacc(
        "TRN2", target_bir_lowering=False, debug=False, num_devices=NCORES
    )

    # pk: [KP, 2*FREE] bf16 — adj terms in [:, :FREE], one-hot selector
    # blocks in [:, FREE:]
    pk = nc.dram_tensor("pk", [KP, 2 * FREE], bf16, kind="ExternalInput").ap()
    # ckf: [128, 1024] bf16 — node terms | WjT terms | WiT | diag-scaled node
    ckf = nc.dram_tensor("ckf", [NIN, CKF_COLS], bf16, kind="ExternalInput").ap()
    if PAD_IN:
        nc.dram_tensor("padx", [1, PAD_IN], mybir.dt.uint8, kind="ExternalInput")
    if LAYOUT == "po":
        out = nc.dram_tensor("out", [CH * NOUT, FREE], f32, kind="ExternalOutput").ap()
    else:
        out = nc.dram_tensor("out", [NOUT, RPC * N], f32, kind="ExternalOutput").ap()

    with tile.TileContext(nc) as tc, ExitStack() as ctx:
        const = ctx.enter_context(tc.tile_pool(name="const", bufs=1))
        psum = ctx.enter_context(tc.tile_pool(name="psum", bufs=2, space="PSUM"))
        outp = ctx.enter_context(tc.tile_pool(name="outp", bufs=OUT_BUFS))

        # ckf (sync ring) and pk (scalar ring) load concurrently; the u
        # matmul chain off ckf is the critical path to the first store.
        ckf_sb = const.tile([NIN, CKF_COLS], bf16)
        nc.sync.dma_start(out=ckf_sb[:], in_=ckf)
        pk_sb = const.tile([KP, 2 * FREE], bf16)
        nc.scalar.dma_start(out=pk_sb[:], in_=pk)

        n0 = ckf_sb[:, 0:N]
        n1 = ckf_sb[:, N : 2 * N]
        wj0 = ckf_sb[:, 2 * N : 2 * N + NOUT]
        wj1 = ckf_sb[:, 2 * N + NOUT : 2 * N + 2 * NOUT]
        wi0 = ckf_sb[:, 2 * N + 2 * NOUT : 2 * N + 3 * NOUT]
        nd0 = ckf_sb[:, 2 * N + 3 * NOUT : 2 * N + 4 * NOUT]

        # u = Wj @ node_r -> [nout, N], via 2-term bf16 splits (err ~2^-17)
        ps_u = psum.tile([NOUT, N], f32, tag="mm")
        nc.tensor.matmul(ps_u[:], lhsT=wj0, rhs=n0, start=True, stop=False)
        nc.tensor.matmul(ps_u[:], lhsT=wj0, rhs=n1, start=False, stop=False)
        nc.tensor.matmul(ps_u[:], lhsT=wj1, rhs=n0, start=False, stop=True)
        u_sb = const.tile([NOUT, N], f32)
        nc.vector.tensor_copy(u_sb[:], ps_u[:])

        # dv[o,l] = adj_diag[l] * (Wi @ node_r)[o,l]; the diag scaling is
        # folded into nd on the host, so this is one matmul + copy.
        ps_dv = psum.tile([NOUT, RPC], f32, tag="mm")
        nc.tensor.matmul(ps_dv[:], lhsT=wi0, rhs=nd0, start=True, stop=True)
        dv_sb = const.tile([NOUT, RPC], f32)
        nc.scalar.copy(dv_sb[:], ps_dv[:])

        def patch(o_ap, base, p, c0, w):
            # diagonal of local row l=8p+k sits at chunk-free offset 8p+k*257
            k0 = max(0, -(-(c0 - RCH * p) // 257))
            k1 = min(RCH - 1, (c0 + w - 1 - RCH * p) // 257)
            if k0 <= k1:
                nc.scalar.copy(
                    o_ap[
                        :,
                        base + RCH * p + 257 * k0 - c0 : base
                        + RCH * p
                        + 257 * k1
                        - c0
                        + 1 : 257,
                    ],
                    dv_sb[:, RCH * p + k0 : RCH * p + k1 + 1],
                )

        def dst_of(p, c0, w):
            if LAYOUT == "po":
                return out[NOUT * p : NOUT * (p + 1), c0 : c0 + w]
            return out[:, FREE * p + c0 : FREE * p + c0 + w]

        ui = 0
        # fine-grained early chunks: per-unit PSUM + SBUF tiles so the first
        # store leaves as soon as the first 512 columns are multiplied
        fine = [(p, c0, w) for p, c0, w in _units() if p < len(SPLIT)]
        nfine = len({p for p, _, _ in fine})
        for p, c0, w in fine:
            ps = psum.tile([NOUT, w], f32, tag="mm", name=f"ps_{p}_{c0}")
            lhs = pk_sb[:, FREE + NOUT * p : FREE + NOUT * (p + 1)]
            for q in range(w // 512):
                nc.tensor.matmul(
                    ps[:, 512 * q : 512 * (q + 1)],
                    lhsT=lhs,
                    rhs=pk_sb[:, c0 + 512 * q : c0 + 512 * (q + 1)],
                    start=True,
                    stop=True,
                )
            o_sb = outp.tile([NOUT, w], f32, tag="osb_s", bufs=6, name=f"o_{p}_{c0}")
            k = w // N
            u_rep = u_sb[:].unsqueeze(1).broadcast_to([NOUT, k, N])
            nc.vector.tensor_mul(
                o_sb[:].rearrange("p (k j) -> p k j", k=k),
                ps[:].rearrange("p (k j) -> p k j", k=k),
                u_rep,
            )
            patch(o_sb, 0, p, c0, w)
            eng = nc.sync if ui % 2 == 0 else nc.scalar
            eng.dma_start(out=dst_of(p, c0, w), in_=o_sb[:])
            ui += 1

        # steady state: identical to the proven baseline — per-chunk PSUM
        # tiles, 2-chunk (2 MiB) group stores alternating HWDGE rings
        u_rep8 = u_sb[:].unsqueeze(1).broadcast_to([NOUT, RCH, N])
        p = nfine
        while p < CH:
            gsz = min(2, CH - p)
            o_sb = outp.tile(
                [NOUT, gsz * FREE], f32, tag="osb", name=f"o_g{p}"
            )
            p0 = p
            for g in range(gsz):
                ps_b = psum.tile([NOUT, FREE], f32, tag="mm", name=f"ps_b{p}")
                lhs = pk_sb[:, FREE + NOUT * p : FREE + NOUT * (p + 1)]
                for q in range(FREE // 512):
                    sl = slice(512 * q, 512 * (q + 1))
                    nc.tensor.matmul(
                        ps_b[:, sl], lhsT=lhs, rhs=pk_sb[:, sl], start=True, stop=True
                    )
                nc.vector.tensor_mul(
                    o_sb[:, g * FREE : (g + 1) * FREE].rearrange(
                        "p (k j) -> p k j", k=RCH
                    ),
                    ps_b[:].rearrange("p (k j) -> p k j", k=RCH),
                    u_rep8,
                )
                patch(o_sb, g * FREE, p, 0, FREE)
                p += 1
            eng = nc.sync if ui % 2 == 0 else nc.scalar
            if LAYOUT == "po":
                dst = out[NOUT * p0 : NOUT * p, 0:FREE].rearrange(
                    "(g o) w -> o g w", g=gsz
                )
                eng.dma_start(
                    out=dst, in_=o_sb[:].rearrange("p (g w) -> p g w", g=gsz)
                )
            else:
                eng.dma_start(out=out[:, FREE * p0 : FREE * p], in_=o_sb[:])
            ui += 1

    nc.compile()
    _cached[key] = nc
    return nc


def _split_terms(x, nterms):
    """Split fp32 array into bf16 terms whose fp32 sum approximates x.
    1 term has <=2^-9 relative error, 2 terms <=2^-18, 3 terms exact."""
    import ml_dtypes

    terms = []
    r = x
    for _ in range(nterms):
        t = r.astype(ml_dtypes.bfloat16)
        terms.append(t)
        r = (r - t.astype(np.float32)).astype(np.float32)
    return terms


def _in_maps(adj, node, Wi, Wj):
    import ml_dtypes

    bf16 = ml_dtypes.bfloat16
    sel = np.zeros((KP, CH * NOUT), bf16)
    for p in range(CH):
        for t in range(NTERMS):
            sel[CH * t + p, NOUT * p : NOUT * (p + 1)] = 1.0
    wj_t = _split_terms(Wj.T, 2)
    wi0 = Wi.T.astype(bf16)
    maps = []
    for c in range(NCORES):
        b, h = divmod(c, 2)
        r0 = RPC * h
        a = adj[b, 0, r0 : r0 + RPC, :]
        diag_row = a[np.arange(RPC), r0 + np.arange(RPC)]
        if h:
            ar = np.roll(a, -r0, axis=1)
            noder = np.roll(node[b], -r0, axis=1)
        else:
            ar = a
            noder = node[b]
        pk = np.empty((KP, 2 * FREE), bf16)
        terms = _split_terms(ar.reshape(CH, FREE), NTERMS)
        for t in range(NTERMS):
            pk[CH * t : CH * (t + 1), 0:FREE] = terms[t]
        pk[:, FREE:] = sel
        n_t = _split_terms(noder, 2)
        ckf = np.empty((NIN, CKF_COLS), bf16)
        ckf[:, 0:N] = n_t[0]
        ckf[:, N : 2 * N] = n_t[1]
        ckf[:, 2 * N : 2 * N + NOUT] = wj_t[0]
        ckf[:, 2 * N + NOUT : 2 * N + 2 * NOUT] = wj_t[1]
        ckf[:, 2 * N + 2 * NOUT : 2 * N + 3 * NOUT] = wi0
        ckf[:, 2 * N + 3 * NOUT :] = (
            noder[:, 0:RPC] * diag_row[None, :].astype(np.float32)
        ).astype(bf16)
        m = {"pk": pk, "ckf": ckf}
        if PAD_IN:
            m["padx"] = np.zeros((1, PAD_IN), np.uint8)
        maps.append(m)
    return maps


def kernel(**inputs):
    global last_results
    adj = np.asarray(inputs["adj"], dtype=np.float32)
    node = np.asarray(inputs["node"], dtype=np.float32)
    Wi = np.asarray(inputs["Wi"], dtype=np.float32)
    Wj = np.asarray(inputs["Wj"], dtype=np.float32)

    from concourse.bass_utils import run_bass_kernel_spmd

    nc = _build_nc()
    res = run_bass_kernel_spmd(nc, _in_maps(adj, node, Wi, Wj), list(range(NCORES)))
    last_results = res

    out = np.empty((B, NOUT, N, N), np.float32)
    for c in range(NCORES):
        b, h = divmod(c, 2)
        co = res.results[c]["out"]
        if LAYOUT == "po":
            co = np.ascontiguousarray(
                co.reshape(CH, NOUT, RCH, N).transpose(1, 0, 2, 3)
            ).reshape(NOUT, RPC, N)
        else:
            co = co.reshape(NOUT, RPC, N)
        if h:
            co = np.roll(co, RPC * h, axis=2)
        out[b, :, RPC * h : RPC * (h + 1), :] = co
    return out


# revision 13
# speedup vs baseline: 1.1684x; 1.1192x over previous
"""NodeConv kernel for 8 Trainium2 NeuronCores — ORIGINAL BASELINE (control run).

See kernel_hybrid.py for the optimized variant under development.
"""

import os

import numpy as np

NCORES = 8
B, N, NIN, NOUT = 4, 256, 128, 128
RPC = 128          # rows per core
CH = 16            # chunks per core
RCH = 8            # rows per chunk
FREE = RCH * N     # 2048 free elems per chunk

NTERMS = int(os.environ.get("NODECONV_NTERMS", "3"))   # bf16 terms (2 or 3)
SG = int(os.environ.get("NODECONV_SG", "2"))           # chunks per store group
OUT_BUFS = int(os.environ.get("NODECONV_OUT_BUFS", "4"))
_G = [SG] * ((CH - 2) // SG) + [1, 1] if SG > 1 else [1] * CH
assert sum(_G) == CH
_GP_CHUNKS = {
    int(x)
    for x in os.environ.get("NODECONV_GP_CHUNKS", "").split(",")
    if x != ""
}

KP = CH * NTERMS   # contraction partitions of the broadcast matmul

_cached = {}

last_results = None  # BassKernelResults of the most recent kernel() call


def _build_nc():
    key = (NTERMS, SG, OUT_BUFS)
    if key in _cached:
        return _cached[key]

    from contextlib import ExitStack

    import concourse.tile as tile
    from concourse import bacc, mybir

    f32 = mybir.dt.float32
    bf16 = mybir.dt.bfloat16

    nc = bacc.Bacc(
        "TRN2", target_bir_lowering=False, debug=False, num_devices=NCORES
    )

    pk = nc.dram_tensor("pk", [KP, 2 * FREE], bf16, kind="ExternalInput").ap()
    ckf = nc.dram_tensor("ckf", [NIN, N + 2 * NOUT], f32, kind="ExternalInput").ap()
    dsz = nc.dram_tensor("dsz", [CH, 2 * RPC], f32, kind="ExternalInput").ap()
    out = nc.dram_tensor("out", [NOUT, RPC * N], f32, kind="ExternalOutput").ap()

    with tile.TileContext(nc) as tc, ExitStack() as ctx:
        const = ctx.enter_context(tc.tile_pool(name="const", bufs=1))
        psum = ctx.enter_context(tc.tile_pool(name="psum", bufs=2, space="PSUM"))
        outp = ctx.enter_context(tc.tile_pool(name="outp", bufs=OUT_BUFS))
        stage = (
            ctx.enter_context(tc.tile_pool(name="stage", bufs=2))
            if _GP_CHUNKS
            else None
        )

        ckf_sb = const.tile([NIN, N + 2 * NOUT], f32)
        nc.sync.dma_start(out=ckf_sb[:], in_=ckf)
        pk_sb = const.tile([KP, 2 * FREE], bf16)
        nc.sync.dma_start(out=pk_sb[:], in_=pk)
        dsz_sb = const.tile([CH, 2 * RPC], f32)
        nc.scalar.dma_start(out=dsz_sb[:], in_=dsz)

        node_sb = ckf_sb[:, 0:N]
        wit_sb = ckf_sb[:, N : N + NOUT]
        wjt_sb = ckf_sb[:, N + NOUT : N + 2 * NOUT]
        diag_sb = dsz_sb[:, 0:RPC]
        selz_sb = dsz_sb[:, RPC : 2 * RPC]

        ps_u = psum.tile([NOUT, N], f32, tag="mm")
        nc.tensor.matmul(ps_u[:], lhsT=wjt_sb, rhs=node_sb, start=True, stop=True)
        u_sb = const.tile([NOUT, N], f32)
        nc.scalar.copy(u_sb[:], ps_u[:])

        ps_v = psum.tile([NOUT, RPC], f32, tag="mm")
        nc.tensor.matmul(
            ps_v[:], lhsT=wit_sb, rhs=node_sb[:, 0:RPC], start=True, stop=True
        )
        v_sb = const.tile([NOUT, RPC], f32)
        nc.scalar.copy(v_sb[:], ps_v[:])

        ps_d = psum.tile([NOUT, RPC], f32, tag="mm")
        nc.tensor.matmul(ps_d[:], lhsT=selz_sb, rhs=diag_sb, start=True, stop=True)
        dv_sb = const.tile([NOUT, RPC], f32)
        nc.vector.tensor_mul(dv_sb[:], ps_d[:], v_sb[:])

        u_rep = u_sb[:].unsqueeze(1).broadcast_to([NOUT, RCH, N])

        p = 0
        for gi, gsz in enumerate(_G):
            o_sb = outp.tile([NOUT, gsz * FREE], f32, tag="osb")
            p0 = p
            for g in range(gsz):
                ps_b = psum.tile([NOUT, FREE], f32, tag="mm")
                lhs = pk_sb[:, FREE + NOUT * p : FREE + NOUT * (p + 1)]
                for q in range(FREE // 512):
                    sl = slice(512 * q, 512 * (q + 1))
                    nc.tensor.matmul(
                        ps_b[:, sl], lhsT=lhs, rhs=pk_sb[:, sl], start=True, stop=True
                    )
                o_view = o_sb[:, g * FREE : (g + 1) * FREE].rearrange(
                    "p (k j) -> p k j", k=RCH
                )
                if p in _GP_CHUNKS:
                    st_sb = stage.tile([NOUT, FREE], f32, tag="st")
                    nc.scalar.copy(st_sb[:], ps_b[:])
                    nc.gpsimd.tensor_mul(
                        o_view, st_sb[:].rearrange("p (k j) -> p k j", k=RCH), u_rep
                    )
                else:
                    nc.vector.tensor_mul(
                        o_view, ps_b[:].rearrange("p (k j) -> p k j", k=RCH), u_rep
                    )
                nc.scalar.copy(
                    o_sb[
                        :,
                        g * FREE + RCH * p : g * FREE
                        + RCH * p
                        + (RCH - 1) * (N + 1)
                        + 1 : N + 1,
                    ],
                    dv_sb[:, RCH * p : RCH * (p + 1)],
                )
                p += 1
            eng = nc.sync if gi % 2 == 0 else nc.scalar
            eng.dma_start(out=out[:, FREE * p0 : FREE * p], in_=o_sb[:])

    nc.compile()
    _cached[key] = nc
    return nc


def _split_terms(x, nterms):
    import ml_dtypes

    terms = []
    r = x
    for _ in range(nterms):
        t = r.astype(ml_dtypes.bfloat16)
        terms.append(t)
        r = (r - t.astype(np.float32)).astype(np.float32)
    return terms


def _in_maps(adj, node, Wi, Wj):
    import ml_dtypes

    bf16 = ml_dtypes.bfloat16
    sel = np.zeros((KP, CH * NOUT), bf16)
    for p in range(CH):
        for t in range(NTERMS):
            sel[CH * t + p, NOUT * p : NOUT * (p + 1)] = 1.0
    dszz = np.zeros((CH, 2 * RPC), np.float32)
    dszz[0, RPC : 2 * RPC] = 1.0
    ckf = np.empty((NIN, N + 2 * NOUT), np.float32)
    ckf[:, N : N + NOUT] = Wi.T
    ckf[:, N + NOUT :] = Wj.T
    maps = []
    for c in range(NCORES):
        b, h = divmod(c, 2)
        r0 = RPC * h
        a = adj[b, 0, r0 : r0 + RPC, :]
        dsz = dszz.copy()
        dsz[0, 0:RPC] = a[np.arange(RPC), r0 + np.arange(RPC)]
        if h:
            ar = np.roll(a, -r0, axis=1)
            noder = np.roll(node[b], -r0, axis=1)
        else:
            ar = a
            noder = node[b]
        pk = np.empty((KP, 2 * FREE), bf16)
        terms = _split_terms(ar.reshape(CH, FREE), NTERMS)
        for t in range(NTERMS):
            pk[CH * t : CH * (t + 1), 0:FREE] = terms[t]
        pk[:, FREE:] = sel
        m_ckf = ckf.copy()
        m_ckf[:, 0:N] = noder
        maps.append({"pk": pk, "ckf": m_ckf, "dsz": dsz})
    return maps


def kernel(**inputs):
    global last_results
    adj = np.asarray(inputs["adj"], dtype=np.float32)
    node = np.asarray(inputs["node"], dtype=np.float32)
    Wi = np.asarray(inputs["Wi"], dtype=np.float32)
    Wj = np.asarray(inputs["Wj"], dtype=np.float32)

    from concourse.bass_utils import run_bass_kernel_spmd

    nc = _build_nc()
    res = run_bass_kernel_spmd(nc, _in_maps(adj, node, Wi, Wj), list(range(NCORES)))
    last_results = res

    out = np.empty((B, NOUT, N, N), np.float32)
    for c in range(NCORES):
        b, h = divmod(c, 2)
        co = res.results[c]["out"].reshape(NOUT, RPC, N)
        if h:
            co = np.roll(co, RPC * h, axis=2)
        out[b, :, RPC * h : RPC * (h + 1), :] = co
    return out


# revision 17
# speedup vs baseline: 1.1841x; 1.0134x over previous
"""NodeConv kernel for 8 Trainium2 NeuronCores.

Reference computes, for adj [B,1,N,N], node [B,nin,N], Wi/Wj [nout,nin]:
    x  = node[:, :, None, :] * adj          # [B,nin,N,N]
    yi = einsum('oc,bcij->boij', Wi, x)
    yj = einsum('oc,bcij->boij', Wj, x)
    out = I * yi + (1-I) * yj

Because adj[b,i,j] does not depend on the contraction channel c, the
contraction factors out:
    off-diag: out[b,o,i,j] = adj[b,i,j] * (Wj @ node[b])[o,j]
    diag:     out[b,o,j,j] = adj[b,j,j] * (Wi @ node[b])[o,j]

So per batch we need one tiny matmul u = Wj@node, a broadcast multiply
out[o,i,j] = adj[i,j]*u[o,j], and a diagonal patch with
dv = Wi @ (node[:, :128] * adj_diag) (the diag scaling is folded into
the host-prepared input, so dv is a single matmul).

The 128 MiB output write is the memory roofline; per core the 16 MiB
store drains at ~425 GB/s (~39.5 us) with the 16 HW queues saturated,
so total time = (time until the first store hits the queues) + 39.5 us.
Lead-in minimization:
  - adj ships as ONE bf16 term (2^-9 relative error, gate is 2e-2),
    so the pk input load is 128 KiB instead of 384 KiB
  - ckf is all-bf16 (node/weights as 2-term bf16 splits, u error
    ~2^-17): 256 KiB, and its u-matmul chain is 3 fast bf16 matmuls
  - ckf loads on the sync HWDGE ring while pk loads on the scalar ring
  - chunk 0 is processed in four 512-column units and chunk 1 in two
    1024-column units, EACH with its own PSUM tile + SBUF tile (the
    tile framework tracks dependencies per tile, not per subrange), so
    the first 256 KiB store is in flight ~6 us earlier than the
    baseline's first 2 MiB group store

Sharding: core c handles batch b=c//2, row half h=c%2 (128 rows). Odd
halves get their columns rolled by -128 on the host so the diagonal of
local row l sits at local column l on every core -> one SPMD program;
the host rolls the output back while gathering.

Per-core device program:
  - u = Wj @ node_r (PE, 3 accumulating bf16 matmuls), DVE copies it
    to SBUF; dv = Wi @ nd (1 bf16 matmul), ScalarE copies it
  - per store unit (chunk p, cols [c0,c0+w)): PE broadcasts the adj
    rows to all 128 partitions with one-hot-selector matmuls (one per
    512 cols / PSUM bank); DVE multiplies by u broadcast along the row
    dim; ScalarE patches the diagonal elements via a stride-257 view;
    the store alternates between the two HWDGE rings.
    NODECONV_LAYOUT=po stores chunk-major (contiguous DRAM blocks) and
    the host transposes while gathering.
"""

import os

import numpy as np

NCORES = 8
B, N, NIN, NOUT = 4, 256, 128, 128
RPC = 128          # rows per core
CH = 16            # chunks per core
RCH = 8            # rows per chunk
FREE = RCH * N     # 2048 free elems per chunk

NTERMS = int(os.environ.get("NODECONV_NTERMS", "1"))   # bf16 terms (1, 2 or 3)
OUT_BUFS = int(os.environ.get("NODECONV_OUT_BUFS", "4"))
LAYOUT = os.environ.get("NODECONV_LAYOUT", "oc")       # oc: [NOUT, RPC*N]; po: chunk-major
# how many store units the first chunks split into ("4,2": chunk0 -> 4x512,
# chunk1 -> 2x1024, rest whole)
SPLIT = [int(x) for x in os.environ.get("NODECONV_SPLIT", "4,2").split(",") if x]
# bytes of dummy DRAM input declared before `out` (shifts its placement)
PAD_IN = int(os.environ.get("NODECONV_PAD_IN", "0"))
# 2: u from 2-term bf16 splits (3 accumulating matmuls); 1: single matmul
U_TERMS = int(os.environ.get("NODECONV_U_TERMS", "2"))

KP = CH * NTERMS   # contraction partitions of the broadcast matmul
CKF_COLS = 2 * N + 4 * NOUT   # n0 n1 | wj0 wj1 | wi0 | nd0

_cached = {}

last_results = None  # BassKernelResults of the most recent kernel() call


def _units():
    """(chunk, col0, width) store units; early chunks split finer."""
    units = []
    for p in range(CH):
        nsub = SPLIT[p] if p < len(SPLIT) else 1
        w = FREE // nsub
        for s in range(nsub):
            units.append((p, s * w, w))
    return units


def _build_nc():
    key = (NTERMS, OUT_BUFS, LAYOUT, tuple(SPLIT), PAD_IN, U_TERMS)
    if key in _cached:
        return _cached[key]

    from contextlib import ExitStack

    import concourse.tile as tile
    from concourse import bacc, mybir

    f32 = mybir.dt.float32
    bf16 = mybir.dt.bfloat16

    nc = bacc.Bacc(
        "TRN2", target_bir_lowering=False, debug=False, num_devices=NCORES
    )

    # pk: [KP, 2*FREE] bf16 — adj terms in [:, :FREE], one-hot selector
    # blocks in [:, FREE:]
    pk = nc.dram_tensor("pk", [KP, 2 * FREE], bf16, kind="ExternalInput").ap()
    # ckf: [128, 1024] bf16 — node terms | WjT terms | WiT | diag-scaled node
    ckf = nc.dram_tensor("ckf", [NIN, CKF_COLS], bf16, kind="ExternalInput").ap()
    if PAD_IN:
        nc.dram_tensor("padx", [1, PAD_IN], mybir.dt.uint8, kind="ExternalInput")
    if LAYOUT == "po":
        out = nc.dram_tensor("out", [CH * NOUT, FREE], f32, kind="ExternalOutput").ap()
    else:
        out = nc.dram_tensor("out", [NOUT, RPC * N], f32, kind="ExternalOutput").ap()

    with tile.TileContext(nc) as tc, ExitStack() as ctx:
        const = ctx.enter_context(tc.tile_pool(name="const", bufs=1))
        psum = ctx.enter_context(tc.tile_pool(name="psum", bufs=2, space="PSUM"))
        outp = ctx.enter_context(tc.tile_pool(name="outp", bufs=OUT_BUFS))

        # ckf (sync ring) and pk (scalar ring) load concurrently; the u
        # matmul chain off ckf is the critical path to the first store.
        ckf_sb = const.tile([NIN, CKF_COLS], bf16)
        nc.sync.dma_start(out=ckf_sb[:], in_=ckf)
        pk_sb = const.tile([KP, 2 * FREE], bf16)
        nc.scalar.dma_start(out=pk_sb[:], in_=pk)

        n0 = ckf_sb[:, 0:N]
        n1 = ckf_sb[:, N : 2 * N]
        wj0 = ckf_sb[:, 2 * N : 2 * N + NOUT]
        wj1 = ckf_sb[:, 2 * N + NOUT : 2 * N + 2 * NOUT]
        wi0 = ckf_sb[:, 2 * N + 2 * NOUT : 2 * N + 3 * NOUT]
        nd0 = ckf_sb[:, 2 * N + 3 * NOUT : 2 * N + 4 * NOUT]

        # u = Wj @ node_r -> [nout, N], via 2-term bf16 splits (err ~2^-17)
        ps_u = psum.tile([NOUT, N], f32, tag="mm")
        if U_TERMS == 2:
            nc.tensor.matmul(ps_u[:], lhsT=wj0, rhs=n0, start=True, stop=False)
            nc.tensor.matmul(ps_u[:], lhsT=wj0, rhs=n1, start=False, stop=False)
            nc.tensor.matmul(ps_u[:], lhsT=wj1, rhs=n0, start=False, stop=True)
        else:
            nc.tensor.matmul(ps_u[:], lhsT=wj0, rhs=n0, start=True, stop=True)
        u_sb = const.tile([NOUT, N], f32)
        nc.vector.tensor_copy(u_sb[:], ps_u[:])

        # dv[o,l] = adj_diag[l] * (Wi @ node_r)[o,l]; the diag scaling is
        # folded into nd on the host, so this is one matmul + copy.
        ps_dv = psum.tile([NOUT, RPC], f32, tag="mm")
        nc.tensor.matmul(ps_dv[:], lhsT=wi0, rhs=nd0, start=True, stop=True)
        dv_sb = const.tile([NOUT, RPC], f32)
        nc.scalar.copy(dv_sb[:], ps_dv[:])

        def patch(o_ap, base, p, c0, w):
            # diagonal of local row l=8p+k sits at chunk-free offset 8p+k*257
            k0 = max(0, -(-(c0 - RCH * p) // 257))
            k1 = min(RCH - 1, (c0 + w - 1 - RCH * p) // 257)
            if k0 <= k1:
                nc.scalar.copy(
                    o_ap[
                        :,
                        base + RCH * p + 257 * k0 - c0 : base
                        + RCH * p
                        + 257 * k1
                        - c0
                        + 1 : 257,
                    ],
                    dv_sb[:, RCH * p + k0 : RCH * p + k1 + 1],
                )

        def dst_of(p, c0, w):
            if LAYOUT == "po":
                return out[NOUT * p : NOUT * (p + 1), c0 : c0 + w]
            return out[:, FREE * p + c0 : FREE * p + c0 + w]

        ui = 0
        # fine-grained early chunks: per-unit PSUM + SBUF tiles so the first
        # store leaves as soon as the first 512 columns are multiplied
        fine = [(p, c0, w) for p, c0, w in _units() if p < len(SPLIT)]
        nfine = len({p for p, _, _ in fine})
        for p, c0, w in fine:
            ps = psum.tile([NOUT, w], f32, tag="mm", name=f"ps_{p}_{c0}")
            lhs = pk_sb[:, FREE + NOUT * p : FREE + NOUT * (p + 1)]
            for q in range(w // 512):
                nc.tensor.matmul(
                    ps[:, 512 * q : 512 * (q + 1)],
                    lhsT=lhs,
                    rhs=pk_sb[:, c0 + 512 * q : c0 + 512 * (q + 1)],
                    start=True,
                    stop=True,
                )
            o_sb = outp.tile([NOUT, w], f32, tag="osb_s", bufs=6, name=f"o_{p}_{c0}")
            k = w // N
            u_rep = u_sb[:].unsqueeze(1).broadcast_to([NOUT, k, N])
            nc.vector.tensor_mul(
                o_sb[:].rearrange("p (k j) -> p k j", k=k),
                ps[:].rearrange("p (k j) -> p k j", k=k),
                u_rep,
            )
            patch(o_sb, 0, p, c0, w)
            eng = nc.sync if ui % 2 == 0 else nc.scalar
            eng.dma_start(out=dst_of(p, c0, w), in_=o_sb[:])
            ui += 1

        # steady state: identical to the proven baseline — per-chunk PSUM
        # tiles, 2-chunk (2 MiB) group stores alternating HWDGE rings
        u_rep8 = u_sb[:].unsqueeze(1).broadcast_to([NOUT, RCH, N])
        p = nfine
        while p < CH:
            gsz = min(2, CH - p)
            o_sb = outp.tile(
                [NOUT, gsz * FREE], f32, tag="osb", name=f"o_g{p}"
            )
            p0 = p
            for g in range(gsz):
                ps_b = psum.tile([NOUT, FREE], f32, tag="mm", name=f"ps_b{p}")
                lhs = pk_sb[:, FREE + NOUT * p : FREE + NOUT * (p + 1)]
                for q in range(FREE // 512):
                    sl = slice(512 * q, 512 * (q + 1))
                    nc.tensor.matmul(
                        ps_b[:, sl], lhsT=lhs, rhs=pk_sb[:, sl], start=True, stop=True
                    )
                nc.vector.tensor_mul(
                    o_sb[:, g * FREE : (g + 1) * FREE].rearrange(
                        "p (k j) -> p k j", k=RCH
                    ),
                    ps_b[:].rearrange("p (k j) -> p k j", k=RCH),
                    u_rep8,
                )
                patch(o_sb, g * FREE, p, 0, FREE)
                p += 1
            eng = nc.sync if ui % 2 == 0 else nc.scalar
            if LAYOUT == "po":
                dst = out[NOUT * p0 : NOUT * p, 0:FREE].rearrange(
                    "(g o) w -> o g w", g=gsz
                )
                eng.dma_start(
                    out=dst, in_=o_sb[:].rearrange("p (g w) -> p g w", g=gsz)
                )
            else:
                eng.dma_start(out=out[:, FREE * p0 : FREE * p], in_=o_sb[:])
            ui += 1

    nc.compile()
    _cached[key] = nc
    return nc


def _split_terms(x, nterms):
    """Split fp32 array into bf16 terms whose fp32 sum approximates x.
    1 term has <=2^-9 relative error, 2 terms <=2^-18, 3 terms exact."""
    import ml_dtypes

    terms = []
    r = x
    for _ in range(nterms):
        t = r.astype(ml_dtypes.bfloat16)
        terms.append(t)
        r = (r - t.astype(np.float32)).astype(np.float32)
    return terms


def _in_maps(adj, node, Wi, Wj):
    import ml_dtypes

    bf16 = ml_dtypes.bfloat16
    sel = np.zeros((KP, CH * NOUT), bf16)
    for p in range(CH):
        for t in range(NTERMS):
            sel[CH * t + p, NOUT * p : NOUT * (p + 1)] = 1.0
    wj_t = _split_terms(Wj.T, 2)
    wi0 = Wi.T.astype(bf16)
    maps = []
    for c in range(NCORES):
        b, h = divmod(c, 2)
        r0 = RPC * h
        a = adj[b, 0, r0 : r0 + RPC, :]
        diag_row = a[np.arange(RPC), r0 + np.arange(RPC)]
        if h:
            ar = np.roll(a, -r0, axis=1)
            noder = np.roll(node[b], -r0, axis=1)
        else:
            ar = a
            noder = node[b]
        pk = np.empty((KP, 2 * FREE), bf16)
        terms = _split_terms(ar.reshape(CH, FREE), NTERMS)
        for t in range(NTERMS):
            pk[CH * t : CH * (t + 1), 0:FREE] = terms[t]
        pk[:, FREE:] = sel
        n_t = _split_terms(noder, 2)
        ckf = np.empty((NIN, CKF_COLS), bf16)
        ckf[:, 0:N] = n_t[0]
        ckf[:, N : 2 * N] = n_t[1]
        ckf[:, 2 * N : 2 * N + NOUT] = wj_t[0]
        ckf[:, 2 * N + NOUT : 2 * N + 2 * NOUT] = wj_t[1]
        ckf[:, 2 * N + 2 * NOUT : 2 * N + 3 * NOUT] = wi0
        ckf[:, 2 * N + 3 * NOUT :] = (
            noder[:, 0:RPC] * diag_row[None, :].astype(np.float32)
        ).astype(bf16)
        m = {"pk": pk, "ckf": ckf}
        if PAD_IN:
            m["padx"] = np.zeros((1, PAD_IN), np.uint8)
        maps.append(m)
    return maps


def kernel(**inputs):
    global last_results
    adj = np.asarray(inputs["adj"], dtype=np.float32)
    node = np.asarray(inputs["node"], dtype=np.float32)
    Wi = np.asarray(inputs["Wi"], dtype=np.float32)
    Wj = np.asarray(inputs["Wj"], dtype=np.float32)

    from concourse.bass_utils import run_bass_kernel_spmd

    nc = _build_nc()
    res = run_bass_kernel_spmd(nc, _in_maps(adj, node, Wi, Wj), list(range(NCORES)))
    last_results = res

    out = np.empty((B, NOUT, N, N), np.float32)
    for c in range(NCORES):
        b, h = divmod(c, 2)
        co = res.results[c]["out"]
        if LAYOUT == "po":
            co = np.ascontiguousarray(
                co.reshape(CH, NOUT, RCH, N).transpose(1, 0, 2, 3)
            ).reshape(NOUT, RPC, N)
        else:
            co = co.reshape(NOUT, RPC, N)
        if h:
            co = np.roll(co, RPC * h, axis=2)
        out[b, :, RPC * h : RPC * (h + 1), :] = co
    return out
